# revision 33
# baseline (speedup 1.0000x reference)
"""Trainium2 Bass kernel for the BayesianBeliefNetwork block (8-core SPMD).

Math (see problem reference):
  h    = LayerNorm(x)*gamma + beta                          [B,S,H]
  ev   = sigmoid(mean_s(h @ W_ve.T + b_ve))                 [B,V]
  post = belief-prop(ev, parents, var_emb, cpt_emb)         [B,V]  (5 iters)
  out  = [h, post] @ W_out.T + b_out + x                    [B,S,H]

Sharding: data-parallel over the B*S = 8192 tokens; core c owns 1024 tokens
(batch b = c//2, sequence half c%2).  Parameters replicated.  The per-batch
sequence mean feeding the evidence is completed with a pairwise AllReduce of
the [V] partial logit sums between the two cores sharing a batch.

Device layout: transposed — H on partitions, tokens on the free axis.  The
host supplies x^T per core twice: bf16 (matmul + LN stats operand) and fp32
(exact residual, streamed later).  LayerNorm folds into the matmul epilogue:

  out^T[ho,t] = rstd_t * (W1g^T xbf)[ho,t]          W1g = W_out[:, :H]*gamma
              - r1[ho] * (mu_t*rstd_t)              r1  = W1g.sum(hin)
              + x^T[ho,t]                           (residual, exact fp32)
              + (b_out + W1@beta + W2@post)[ho]     (after belief prop)

LN stats (sum x, sum x^2 over H) come from ones-stationary matmuls into four
row-regions of a single PSUM bank; the per-token rows live as [2,512] (both
token halves on two partitions) so the row math never runs 1024-deep on one
lane.  rstd = Exp(-0.5*Ln(var+eps)) on ScalarE (~1e-5 rel) — the DVE
reciprocal is Newton-iterative and costs ~8us for 1024-deep rows.
Evidence: logits^T = Wve_g^T xbf on PE, then a fused *rstd token-reduce:
  ev[v] = sum_t logits^T[v,t]*rstd_t - rve[v]*sum_t(mu_t*rstd_t).
The cosine's 1/denom parent normalization cancels (scale-invariant), so
belief prop needs no division by parent counts.
"""

import numpy as np
import ml_dtypes

import concourse.bass as bass
import concourse.tile as tile
from concourse import bacc, mybir
from concourse.bass_utils import run_bass_kernel_spmd

F32 = mybir.dt.float32
BF16 = mybir.dt.bfloat16
OP = mybir.AluOpType
AF = mybir.ActivationFunctionType

H = 2048
V = 10
D4 = 512
B = 4
S = 2048
N_CORES = 8
T = (B * S) // N_CORES          # 1024 tokens per core
NCH = H // 128                  # 16 h-chunks
TB = T // 512                   # 2 token halves of 512
LN_EPS = 1e-5
N_ITERS = 5

_PROG = None


def build_program():
    nc = bacc.Bacc("TRN2", target_bir_lowering=False, debug=False,
                   num_devices=N_CORES)

    xbf_d = nc.dram_tensor("xbfT", [H, T], BF16, kind="ExternalInput").ap()
    xT_d = nc.dram_tensor("xT", [H, T], F32, kind="ExternalInput").ap()
    w1_d = nc.dram_tensor("w1t", [H, H], BF16, kind="ExternalInput").ap()
    wve_d = nc.dram_tensor("wve_t", [H, V + 1], BF16, kind="ExternalInput").ap()
    w2t_d = nc.dram_tensor("w2t", [V, H], F32, kind="ExternalInput").ap()
    rve_d = nc.dram_tensor("rve_col", [V, 1], F32, kind="ExternalInput").ap()
    bve_d = nc.dram_tensor("bve_col", [V, 1], F32, kind="ExternalInput").ap()
    hasp_d = nc.dram_tensor("hasp_col", [V, 1], F32, kind="ExternalInput").ap()
    pft_d = nc.dram_tensor("pft", [V, V], F32, kind="ExternalInput").ap()
    var_d = nc.dram_tensor("var_bf", [V, D4], BF16, kind="ExternalInput").ap()
    cpt_d = nc.dram_tensor("cpt", [V, D4], F32, kind="ExternalInput").ap()
    nr1_d = nc.dram_tensor("neg_r1", [128, NCH], F32, kind="ExternalInput").ap()
    bout_d = nc.dram_tensor("bout_col", [128, NCH], F32, kind="ExternalInput").ap()
    out_d = nc.dram_tensor("outT", [H, T], F32, kind="ExternalOutput").ap()

    with tile.TileContext(nc) as tc:
        with (
            tc.tile_pool(name="px", bufs=16) as px,
            tc.tile_pool(name="pxb", bufs=16) as pxb,
            tc.tile_pool(name="pw1", bufs=32) as pw1,
            tc.tile_pool(name="pc", bufs=1) as pc,
            tc.tile_pool(name="ps", bufs=4) as ps,
            tc.tile_pool(name="psum", bufs=3, space="PSUM") as psum,
            tc.tile_pool(name="psbp", bufs=2, space="PSUM") as psbp,
            tc.tile_pool(name="pdram", bufs=1, space="DRAM") as pdram,
        ):
            def scr_tile(name, shape=(128, T), dtype=F32):
                return ps.tile(list(shape), dtype, tag="scr", bufs=4, name=name)

            def acc_psum(name):
                return psum.tile([128, T], F32, tag="acc", bufs=3, name=name)

            def bp_psum(name):
                return psbp.tile([128, 512], F32, tag="bp", bufs=2, name=name)

            # ---- small constants ----
            wve_sb = pc.tile([128, NCH, V + 1], BF16)
            nc.gpsimd.dma_start(out=wve_sb[:],
                                in_=wve_d.rearrange("(c p) v -> p c v", p=128))
            w2t_sb = pc.tile([V, H], F32)
            nc.gpsimd.dma_start(out=w2t_sb[:], in_=w2t_d[:])
            rve_sb = pc.tile([V, 1], F32)
            nc.gpsimd.dma_start(out=rve_sb[:], in_=rve_d[:])
            bve_sb = pc.tile([V, 1], F32)
            nc.gpsimd.dma_start(out=bve_sb[:], in_=bve_d[:])
            hasp_sb = pc.tile([V, 1], F32)
            nc.gpsimd.dma_start(out=hasp_sb[:], in_=hasp_d[:])
            pft_sb = pc.tile([V, V], F32)
            nc.gpsimd.dma_start(out=pft_sb[:], in_=pft_d[:])
            var_sb = pc.tile([V, D4], BF16)
            nc.gpsimd.dma_start(out=var_sb[:], in_=var_d[:])
            cpt_sb = pc.tile([V, D4], F32)
            nc.gpsimd.dma_start(out=cpt_sb[:], in_=cpt_d[:])
            nr1_sb = pc.tile([128, NCH], F32)
            nc.gpsimd.dma_start(out=nr1_sb[:], in_=nr1_d[:])
            bout_sb = pc.tile([128, NCH], F32)
            nc.gpsimd.dma_start(out=bout_sb[:], in_=bout_d[:])
            ones_sb = pc.tile([128, 1], BF16)
            nc.vector.memset(ones_sb[:], 1.0)
            ones_f32 = pc.tile([1, 128], F32)
            nc.vector.memset(ones_f32[:], 1.0)
            eps_ln = pc.tile([1, 1], F32)
            nc.vector.memset(eps_ln[:], LN_EPS)
            eps_pn = pc.tile([V, 1], F32)
            nc.vector.memset(eps_pn[:], 1e-16)
            rstd_bc = pc.tile([128, T], F32)
            murstd_bc = pc.tile([128, T], F32)

            # ---- phase A: xbf loads, squares, LN-stat matmuls ----
            # stats: four 16-matmul accumulation groups into row-regions of a
            # single PSUM bank: row0/1 = sum(x) tb0/tb1, row2/3 = sum(x^2).
            xbfs, xts = [], []
            accs = {}
            st_ps = bp_psum("st_ps")
            st_ps2 = bp_psum("st_ps2")
            w1_tiles = {}

            def emit_wave_dma(w):
                tl = []
                for hin in range(NCH):
                    wt = pw1.tile([128, 512], BF16, tag="w1", bufs=32,
                                  name=f"w1_{w}_{hin}")
                    nc.sync.dma_start(
                        out=wt[:], in_=w1_d[hin * 128:(hin + 1) * 128,
                                            w * 512:(w + 1) * 512])
                    tl.append(wt)
                w1_tiles[w] = tl

            acc0 = acc_psum("acc0")
            accs[0] = acc0
            w1_tiles[0] = []
            for j in range(NCH):
                xbf = pxb.tile([128, T], BF16, tag="xbf", bufs=16, name=f"xbf{j}")
                nc.sync.dma_start(out=xbf[:], in_=xbf_d[j * 128:(j + 1) * 128, :])
                xbfs.append(xbf)
                wt = pw1.tile([128, 512], BF16, tag="w1", bufs=32,
                              name=f"w1_0_{j}")
                nc.sync.dma_start(out=wt[:], in_=w1_d[j * 128:(j + 1) * 128, 0:512])
                w1_tiles[0].append(wt)
                x2 = ps.tile([128, T], BF16, tag="x2", bufs=2, name=f"x2_{j}")
                nc.vector.tensor_mul(x2[:], xbf[:], xbf[:])
                for t in range(TB):
                    sl = slice(t * 512, (t + 1) * 512)
                    st = st_ps if t == 0 else st_ps2
                    # rows 0-9: evidence logits, row 10: sum(x) (ones column)
                    nc.tensor.matmul(st[0:V + 1, :], wve_sb[:, j, :],
                                     xbf[:, sl],
                                     start=(j == 0), stop=(j == NCH - 1),
                                     skip_group_check=True)
                    nc.tensor.matmul(st[32:33, :], ones_sb[:], x2[:, sl],
                                     start=(j == 0), stop=(j == NCH - 1),
                                     skip_group_check=True)
                    # main tile 0, contraction chunk j
                    nc.tensor.matmul(acc0[:, sl], w1_tiles[0][j][:, 0:128],
                                     xbf[:, sl],
                                     start=(j == 0), stop=(j == NCH - 1))
            for j in range(NCH):
                xt = px.tile([128, T], F32, tag="xt", bufs=16, name=f"xt{j}")
                nc.sync.dma_start(out=xt[:], in_=xT_d[j * 128:(j + 1) * 128, :])
                xts.append(xt)
            emit_wave_dma(1)

            # ---- LN stat rows, [1, 1024]: halves gathered into free dim ----
            mu_row = scr_tile("mu_row", (1, T))
            var_row = scr_tile("var_row", (1, T))
            tmp_row = scr_tile("tmp_row", (1, T))
            murstd_row = scr_tile("murstd_row", (1, T))
            lg_sb0 = pc.tile([V + 1, 512], F32, name="lg_sb0")
            nc.vector.tensor_copy(lg_sb0[:], st_ps[0:V + 1, :])
            lg_sb1 = pc.tile([V + 1, 512], F32, name="lg_sb1")
            nc.vector.tensor_copy(lg_sb1[:], st_ps2[0:V + 1, :])
            # engines can only address partitions starting at 0/32/64/96, so
            # hop the sum-x rows (partition 10) to partition 0 via tiny DMAs.
            sx_row = pc.tile([1, T], F32, name="sx_row")
            nc.gpsimd.dma_start(out=sx_row[0:1, 0:512], in_=lg_sb0[V:V + 1, :])
            nc.gpsimd.dma_start(out=sx_row[0:1, 512:], in_=lg_sb1[V:V + 1, :])
            nc.vector.tensor_scalar_mul(mu_row[0:1, :], sx_row[0:1, :], 1.0 / H)
            nc.vector.tensor_scalar_mul(var_row[0:1, 0:512], st_ps[32:33, :],
                                        1.0 / H)
            nc.vector.tensor_scalar_mul(var_row[0:1, 512:], st_ps2[32:33, :],
                                        1.0 / H)
            nc.vector.tensor_mul(tmp_row[0:1, :], mu_row[0:1, :],
                                 mu_row[0:1, :])
            nc.vector.tensor_sub(var_row[0:1, :], var_row[0:1, :],
                                 tmp_row[0:1, :])
            # rstd = (var+eps)^-0.5 via exp(-0.5*ln(.)) on ScalarE (~1e-5 rel);
            # DVE reciprocal is Newton-iterative: ~8us for rows this deep.
            nc.scalar.activation(var_row[0:1, :], var_row[0:1, :], AF.Ln,
                                 bias=eps_ln[:])
            nc.scalar.activation(var_row[0:1, :], var_row[0:1, :], AF.Exp,
                                 bias=0.0, scale=-0.5)
            nc.vector.tensor_mul(murstd_row[0:1, :], mu_row[0:1, :],
                                 var_row[0:1, :])

            # ---- emission helpers ----
            def emit_bc():
                # broadcast rstd / mu*rstd rows into [128, T] via K=1 matmuls
                for t in range(TB):
                    sl = slice(t * 512, (t + 1) * 512)
                    bc_r = bp_psum(f"bc_r{t}")
                    nc.tensor.matmul(bc_r[:], ones_f32[:],
                                     var_row[0:1, sl], start=True, stop=True)
                    nc.scalar.activation(rstd_bc[:, sl], bc_r[:], AF.Copy)
                    bc_m = bp_psum(f"bc_m{t}")
                    nc.tensor.matmul(bc_m[:], ones_f32[:],
                                     murstd_row[0:1, sl], start=True, stop=True)
                    nc.scalar.activation(murstd_bc[:, sl], bc_m[:], AF.Copy)

            def emit_main_tile(j):
                w, jj = j // 4, j % 4
                acc = acc_psum(f"acc{j}")
                for t in range(TB):
                    sl = slice(t * 512, (t + 1) * 512)
                    for hin in range(NCH):
                        nc.tensor.matmul(
                            acc[:, sl],
                            w1_tiles[w][hin][:, jj * 128:(jj + 1) * 128],
                            xbfs[hin][:, sl],
                            start=(hin == 0), stop=(hin == NCH - 1))
                accs[j] = acc

            def emit_evict1(j):
                # xt[j] = psum*rstd + xt[j] + (-r1_j)*murstd
                s2 = scr_tile(f"s2_{j}")
                nc.vector.tensor_scalar_mul(s2[:], murstd_bc[:],
                                            nr1_sb[:, j:j + 1])
                s3 = scr_tile(f"s3_{j}")
                nc.vector.tensor_mul(s3[:], accs.pop(j)[:], rstd_bc[:])
                nc.vector.tensor_add(s3[:], s3[:], s2[:])
                nc.vector.tensor_add(xts[j][:], s3[:], xts[j][:])

            bp = {}

            def emit_ev():
                # token-reduce of the phase-A logits rows against rstd
                ev_acc = pc.tile([V, TB], F32)
                for t, lg_sb in ((0, lg_sb0), (1, lg_sb1)):
                    sl = slice(t * 512, (t + 1) * 512)
                    lg_scr = pc.tile([V, 512], F32, tag="lg", bufs=2,
                                     name=f"lg_scr{t}")
                    nc.vector.scalar_tensor_tensor(
                        out=lg_scr[:], in0=lg_sb[0:V, :], scalar=1.0,
                        in1=rstd_bc[0:V, sl], op0=OP.mult, op1=OP.mult,
                        accum_out=ev_acc[:, t:t + 1])
                rv_scr = pc.tile([V, T], F32)
                rv_acc = pc.tile([V, 1], F32)
                nc.vector.tensor_scalar(rv_scr[:], murstd_bc[0:V, :],
                                        rve_sb[:, 0:1], None, op0=OP.mult,
                                        op1=OP.add, accum_out=rv_acc[:])
                ev_sb = pc.tile([V, 1], F32)
                nc.vector.tensor_add(ev_sb[:], ev_acc[:, 0:1], ev_acc[:, 1:2])
                nc.vector.tensor_sub(ev_sb[:], ev_sb[:], rv_acc[:])

                cc_in = pdram.tile([V, 1], F32)
                cc_out = pdram.tile([V, 1], F32)
                nc.gpsimd.dma_start(out=cc_in[:], in_=ev_sb[:])
                nc.gpsimd.collective_compute(
                    "AllReduce", OP.add,
                    replica_groups=[[0, 1], [2, 3], [4, 5], [6, 7]],
                    ins=[cc_in.opt()], outs=[cc_out.opt()])
                cc_sb = pc.tile([V, 1], F32)
                nc.gpsimd.dma_start(out=cc_sb[:], in_=cc_out[:])
                bp["cc"] = cc_sb

            def emit_bp_pre():
                cc_sb = bp["cc"]
                ev0 = pc.tile([V, 1], F32)
                nc.scalar.activation(ev0[:], cc_sb[:], AF.Sigmoid,
                                     bias=bve_sb[:], scale=1.0 / S)
                m1 = pc.tile([V, 1], F32)
                nc.vector.tensor_scalar(m1[:], ev0[:], 0.1, None, op0=OP.is_gt)
                mask = pc.tile([V, 1], F32)
                nc.vector.tensor_scalar(mask[:], ev0[:], 0.9, None,
                                        op0=OP.is_lt)
                nc.vector.tensor_mul(mask[:], mask[:], m1[:])
                nc.vector.tensor_scalar(mask[:], mask[:], hasp_sb[:, 0:1],
                                        None, op0=OP.mult)
                cn_scr = pc.tile([V, D4], F32)
                icn = pc.tile([V, 1], F32)
                nc.vector.scalar_tensor_tensor(
                    out=cn_scr[:], in0=cpt_sb[:], scalar=1.0, in1=cpt_sb[:],
                    op0=OP.mult, op1=OP.mult, accum_out=icn[:])
                nc.scalar.activation(icn[:], icn[:], AF.Sqrt, bias=0.0)
                nc.vector.reciprocal(icn[:], icn[:])
                probs = pc.tile([V, 1], F32)
                nc.vector.tensor_copy(probs[:], ev0[:])
                bp.update(mask=mask, icn=icn, probs=probs)

            def emit_bp_iter(it):
                mask, icn, probs = bp["mask"], bp["icn"], bp["probs"]
                lhsT = pc.tile([V, V], BF16, name=f"lhsT{it}")
                nc.vector.tensor_scalar(lhsT[:], pft_sb[:], probs[:, 0:1],
                                        None, op0=OP.mult)
                pe_ps = bp_psum(f"pe{it}")
                nc.tensor.matmul(pe_ps[0:V, 0:D4], lhsT[:], var_sb[:],
                                 start=True, stop=True)
                pe_sb = pc.tile([V, D4], F32, tag="bscr", bufs=3,
                                name=f"pe_sb{it}")
                nc.vector.tensor_copy(pe_sb[:], pe_ps[0:V, 0:D4])
                bscr = pc.tile([V, D4], F32, tag="bscr", bufs=3,
                               name=f"bscr{it}")
                dot = pc.tile([V, 1], F32, name=f"dot{it}")
                nc.vector.scalar_tensor_tensor(
                    out=bscr[:], in0=pe_sb[:], scalar=1.0, in1=cpt_sb[:],
                    op0=OP.mult, op1=OP.mult, accum_out=dot[:])
                bscr2 = pc.tile([V, D4], F32, tag="bscr", bufs=3,
                                name=f"bscr2{it}")
                sqn = pc.tile([V, 1], F32, name=f"sqn{it}")
                nc.vector.scalar_tensor_tensor(
                    out=bscr2[:], in0=pe_sb[:], scalar=1.0, in1=pe_sb[:],
                    op0=OP.mult, op1=OP.mult, accum_out=sqn[:])
                # +1e-16 keeps parentless rows (pe == 0) finite; they are
                # masked out of the update anyway (matches the 1e-8 clamp).
                nc.scalar.activation(sqn[:], sqn[:], AF.Sqrt, bias=eps_pn[:])
                ipn = pc.tile([V, 1], F32, name=f"ipn{it}")
                nc.vector.reciprocal(ipn[:], sqn[:])
                s = pc.tile([V, 1], F32, name=f"s{it}")
                nc.vector.tensor_mul(s[:], dot[:], ipn[:])
                nc.vector.tensor_mul(s[:], s[:], icn[:])
                cond = pc.tile([V, 1], F32, name=f"cond{it}")
                nc.scalar.activation(cond[:], s[:], AF.Sigmoid, bias=0.0)
                delta = pc.tile([V, 1], F32, name=f"delta{it}")
                nc.vector.tensor_sub(delta[:], cond[:], probs[:])
                nc.vector.tensor_mul(delta[:], delta[:], mask[:])
                nc.vector.tensor_add(probs[:], probs[:], delta[:])

            def emit_ccol():
                probs = bp["probs"]
                ccol_ps = bp_psum("ccol_ps")
                for c in range(NCH):
                    nc.tensor.matmul(ccol_ps[:, c:c + 1],
                                     w2t_sb[:, c * 128:(c + 1) * 128],
                                     probs[:], start=True, stop=True)
                ccol_sb = pc.tile([128, NCH], F32)
                nc.vector.tensor_add(ccol_sb[:], ccol_ps[:, 0:NCH], bout_sb[:])
                bp["ccol"] = ccol_sb

            def emit_evict2(j):
                nc.vector.tensor_scalar(xts[j][:], xts[j][:],
                                        bp["ccol"][:, j:j + 1], None,
                                        op0=OP.add)
                nc.sync.dma_start(out=out_d[j * 128:(j + 1) * 128, :],
                                  in_=xts[j][:])

            # ---- main emission schedule (tile 0 already ran in phase A) ----
            for j in range(1, NCH):
                if j == 2:
                    emit_wave_dma(2)
                if j == 6:
                    emit_wave_dma(3)
                emit_main_tile(j)
                if j == 2:
                    emit_bc()
                    emit_ev()
                    emit_evict1(0)
                    emit_evict1(1)
                elif 3 <= j <= 14:
                    emit_evict1(j - 1)
                if j == 6:
                    emit_bp_pre()
                if 7 <= j <= 11:
                    emit_bp_iter(j - 7)
                if j == 12:
                    emit_ccol()
                if j == 14:
                    emit_evict1(14)
                    for jj in range(15):
                        emit_evict2(jj)
                if j == 15:
                    emit_evict1(15)
                    emit_evict2(15)

    nc.compile()
    return nc


def _host_prep(hidden_states, gamma, beta, W_ve, b_ve, var_emb, cpt_emb,
               W_out, b_out, parents):
    f32 = np.float32
    x = np.asarray(hidden_states, f32).reshape(B * S, H)
    gamma = np.asarray(gamma, f32)
    beta = np.asarray(beta, f32)
    W_ve = np.asarray(W_ve, f32)
    b_ve = np.asarray(b_ve, f32)
    var_emb = np.asarray(var_emb, f32)
    cpt_emb = np.asarray(cpt_emb, f32)
    W_out = np.asarray(W_out, f32)
    b_out = np.asarray(b_out, f32)
    parents = np.asarray(parents)

    W1 = W_out[:, :H]
    W1g = W1 * gamma[None, :]
    w1t = np.ascontiguousarray(W1g.T).astype(ml_dtypes.bfloat16)
    w2t = np.ascontiguousarray(W_out[:, H:].T)
    Wveg = W_ve * gamma[None, :]
    wve_t = np.ascontiguousarray(
        np.concatenate([Wveg.T, np.ones((H, 1), f32)], axis=1)
    ).astype(ml_dtypes.bfloat16)
    rve_col = Wveg.sum(axis=1).reshape(V, 1).astype(f32)
    bve_col = (b_ve + W_ve @ beta).reshape(V, 1).astype(f32)
    hasp_col = (parents.sum(axis=1) > 0).astype(f32).reshape(V, 1)
    pft = np.ascontiguousarray(parents.T.astype(f32))
    var_bf = var_emb.astype(ml_dtypes.bfloat16)
    cpt = np.ascontiguousarray(cpt_emb, f32)
    neg_r1 = np.ascontiguousarray((-W1g.sum(axis=1)).reshape(NCH, 128).T, f32)
    bout_col = np.ascontiguousarray(
        (b_out + W1 @ beta).reshape(NCH, 128).T, f32)

    shared = dict(w1t=w1t, w2t=w2t, wve_t=wve_t, rve_col=rve_col,
                  bve_col=bve_col, hasp_col=hasp_col, pft=pft, var_bf=var_bf,
                  cpt=cpt, neg_r1=neg_r1, bout_col=bout_col)
    in_maps = []
    for c in range(N_CORES):
        xT = np.ascontiguousarray(x[c * T:(c + 1) * T, :].T)
        in_maps.append(dict(shared, xT=xT,
                            xbfT=xT.astype(ml_dtypes.bfloat16)))
    return in_maps


def kernel(**inputs):
    global _PROG
    if _PROG is None:
        _PROG = build_program()
    nc = _PROG
    in_maps = _host_prep(**inputs)
    res = run_bass_kernel_spmd(nc, in_maps, list(range(N_CORES)))
    out = np.empty((B * S, H), np.float32)
    for c in range(N_CORES):
        out[c * T:(c + 1) * T, :] = res.results[c]["outT"].T
    return out.reshape(B, S, H)


# revision 35
# speedup vs baseline: 1.0200x; 1.0200x over previous
"""Trainium2 Bass kernel for the BayesianBeliefNetwork block (8-core SPMD).

Math (see problem reference):
  h    = LayerNorm(x)*gamma + beta                          [B,S,H]
  ev   = sigmoid(mean_s(h @ W_ve.T + b_ve))                 [B,V]
  post = belief-prop(ev, parents, var_emb, cpt_emb)         [B,V]  (5 iters)
  out  = [h, post] @ W_out.T + b_out + x                    [B,S,H]

Sharding: data-parallel over the B*S = 8192 tokens; core c owns 1024 tokens
(batch b = c//2, sequence half c%2).  Parameters replicated.  The per-batch
sequence mean feeding the evidence is completed with a pairwise AllReduce of
the [V] partial logit sums between the two cores sharing a batch.

Device layout: transposed — H on partitions, tokens on the free axis.  The
host supplies x^T per core twice: bf16 (matmul + LN stats operand) and fp32
(exact residual, streamed later).  LayerNorm folds into the matmul epilogue:

  out^T[ho,t] = rstd_t * (W1g^T xbf)[ho,t]          W1g = W_out[:, :H]*gamma
              - r1[ho] * (mu_t*rstd_t)              r1  = W1g.sum(hin)
              + x^T[ho,t]                           (residual, exact fp32)
              + (b_out + W1@beta + W2@post)[ho]     (after belief prop)

LN stats (sum x, sum x^2 over H) come from ones-stationary matmuls into four
row-regions of a single PSUM bank; the per-token rows live as [2,512] (both
token halves on two partitions) so the row math never runs 1024-deep on one
lane.  rstd = Exp(-0.5*Ln(var+eps)) on ScalarE (~1e-5 rel) — the DVE
reciprocal is Newton-iterative and costs ~8us for 1024-deep rows.
Evidence: logits^T = Wve_g^T xbf on PE, then a fused *rstd token-reduce:
  ev[v] = sum_t logits^T[v,t]*rstd_t - rve[v]*sum_t(mu_t*rstd_t).
The cosine's 1/denom parent normalization cancels (scale-invariant), so
belief prop needs no division by parent counts.
"""

import numpy as np
import ml_dtypes

import concourse.bass as bass
import concourse.tile as tile
from concourse import bacc, mybir
from concourse.bass_utils import run_bass_kernel_spmd

F32 = mybir.dt.float32
BF16 = mybir.dt.bfloat16
OP = mybir.AluOpType
AF = mybir.ActivationFunctionType

H = 2048
V = 10
D4 = 512
B = 4
S = 2048
N_CORES = 8
T = (B * S) // N_CORES          # 1024 tokens per core
NCH = H // 128                  # 16 h-chunks
TB = T // 512                   # 2 token halves of 512
LN_EPS = 1e-5
N_ITERS = 5

_PROG = None


def build_program():
    nc = bacc.Bacc("TRN2", target_bir_lowering=False, debug=False,
                   num_devices=N_CORES)

    xbf_d = nc.dram_tensor("xbfT", [H, T], BF16, kind="ExternalInput").ap()
    xT_d = nc.dram_tensor("xT", [H, T], F32, kind="ExternalInput").ap()
    w1_d = nc.dram_tensor("w1t", [H, H], BF16, kind="ExternalInput").ap()
    wve_d = nc.dram_tensor("wve_t", [H, V + 1], BF16, kind="ExternalInput").ap()
    w2t_d = nc.dram_tensor("w2t", [V, H], F32, kind="ExternalInput").ap()
    rve_d = nc.dram_tensor("rve_col", [V, 1], F32, kind="ExternalInput").ap()
    bve_d = nc.dram_tensor("bve_col", [V, 1], F32, kind="ExternalInput").ap()
    hasp_d = nc.dram_tensor("hasp_col", [V, 1], F32, kind="ExternalInput").ap()
    pft_d = nc.dram_tensor("pft", [V, V], F32, kind="ExternalInput").ap()
    var_d = nc.dram_tensor("var_bf", [V, D4], BF16, kind="ExternalInput").ap()
    cpt_d = nc.dram_tensor("cpt", [V, D4], F32, kind="ExternalInput").ap()
    nr1_d = nc.dram_tensor("neg_r1", [128, NCH], F32, kind="ExternalInput").ap()
    bout_d = nc.dram_tensor("bout_col", [128, NCH], F32, kind="ExternalInput").ap()
    out_d = nc.dram_tensor("outT", [H, T], F32, kind="ExternalOutput").ap()

    with tile.TileContext(nc) as tc:
        with (
            tc.tile_pool(name="px", bufs=16) as px,
            tc.tile_pool(name="pxb", bufs=16) as pxb,
            tc.tile_pool(name="pw1", bufs=32) as pw1,
            tc.tile_pool(name="pc", bufs=1) as pc,
            tc.tile_pool(name="ps", bufs=4) as ps,
            tc.tile_pool(name="psum", bufs=3, space="PSUM") as psum,
            tc.tile_pool(name="psbp", bufs=2, space="PSUM") as psbp,
            tc.tile_pool(name="pdram", bufs=1, space="DRAM") as pdram,
        ):
            def scr_tile(name, shape=(128, T), dtype=F32):
                return ps.tile(list(shape), dtype, tag="scr", bufs=4, name=name)

            def acc_psum(name):
                return psum.tile([128, T], F32, tag="acc", bufs=3, name=name)

            def bp_psum(name):
                return psbp.tile([128, 512], F32, tag="bp", bufs=2, name=name)

            # ---- small constants ----
            # wve first: phase-A logit matmuls wait on it
            wve_sb = pc.tile([128, NCH, V + 1], BF16)
            nc.gpsimd.dma_start(out=wve_sb[:],
                                in_=wve_d.rearrange("(c p) v -> p c v", p=128))
            w2t_sb = pc.tile([V, H], F32)
            nc.gpsimd.dma_start(out=w2t_sb[:], in_=w2t_d[:])
            rve_sb = pc.tile([V, 1], F32)
            nc.gpsimd.dma_start(out=rve_sb[:], in_=rve_d[:])
            bve_sb = pc.tile([V, 1], F32)
            nc.gpsimd.dma_start(out=bve_sb[:], in_=bve_d[:])
            hasp_sb = pc.tile([V, 1], F32)
            nc.gpsimd.dma_start(out=hasp_sb[:], in_=hasp_d[:])
            pft_sb = pc.tile([V, V], F32)
            nc.gpsimd.dma_start(out=pft_sb[:], in_=pft_d[:])
            var_sb = pc.tile([V, D4], BF16)
            nc.gpsimd.dma_start(out=var_sb[:], in_=var_d[:])
            cpt_sb = pc.tile([V, D4], F32)
            nc.gpsimd.dma_start(out=cpt_sb[:], in_=cpt_d[:])
            nr1_sb = pc.tile([128, NCH], F32)
            nc.gpsimd.dma_start(out=nr1_sb[:], in_=nr1_d[:])
            bout_sb = pc.tile([128, NCH], F32)
            nc.gpsimd.dma_start(out=bout_sb[:], in_=bout_d[:])
            ones_sb = pc.tile([128, 1], BF16)
            nc.vector.memset(ones_sb[:], 1.0)
            ones_f32 = pc.tile([1, 128], F32)
            nc.vector.memset(ones_f32[:], 1.0)
            eps_ln = pc.tile([1, 1], F32)
            nc.vector.memset(eps_ln[:], LN_EPS)
            eps_pn = pc.tile([V, 1], F32)
            nc.vector.memset(eps_pn[:], 1e-16)
            rstd_bc = pc.tile([128, T], F32)
            murstd_bc = pc.tile([128, T], F32)

            # ---- phase A: xbf loads, squares, LN-stat matmuls ----
            # stats: four 16-matmul accumulation groups into row-regions of a
            # single PSUM bank: row0/1 = sum(x) tb0/tb1, row2/3 = sum(x^2).
            xbfs, xts = [], []
            accs = {}
            st_ps = bp_psum("st_ps")
            st_ps2 = bp_psum("st_ps2")
            w1_tiles = {}

            def emit_wave_dma(w):
                tl = []
                for hin in range(NCH):
                    wt = pw1.tile([128, 512], BF16, tag="w1", bufs=32,
                                  name=f"w1_{w}_{hin}")
                    nc.sync.dma_start(
                        out=wt[:], in_=w1_d[hin * 128:(hin + 1) * 128,
                                            w * 512:(w + 1) * 512])
                    tl.append(wt)
                w1_tiles[w] = tl

            acc0 = acc_psum("acc0")
            accs[0] = acc0
            w1_tiles[0] = []
            for j in range(NCH):
                xbf = pxb.tile([128, T], BF16, tag="xbf", bufs=16, name=f"xbf{j}")
                nc.sync.dma_start(out=xbf[:], in_=xbf_d[j * 128:(j + 1) * 128, :])
                xbfs.append(xbf)
                wt = pw1.tile([128, 512], BF16, tag="w1", bufs=32,
                              name=f"w1_0_{j}")
                nc.sync.dma_start(out=wt[:], in_=w1_d[j * 128:(j + 1) * 128, 0:512])
                w1_tiles[0].append(wt)
                x2 = ps.tile([128, T], BF16, tag="x2", bufs=2, name=f"x2_{j}")
                nc.vector.tensor_mul(x2[:], xbf[:], xbf[:])
                for t in range(TB):
                    sl = slice(t * 512, (t + 1) * 512)
                    st = st_ps if t == 0 else st_ps2
                    # rows 0-9: evidence logits, row 10: sum(x) (ones column)
                    nc.tensor.matmul(st[0:V + 1, :], wve_sb[:, j, :],
                                     xbf[:, sl],
                                     start=(j == 0), stop=(j == NCH - 1),
                                     skip_group_check=True)
                    nc.tensor.matmul(st[32:33, :], ones_sb[:], x2[:, sl],
                                     start=(j == 0), stop=(j == NCH - 1),
                                     skip_group_check=True)
                    # main tile 0, contraction chunk j
                    nc.tensor.matmul(acc0[:, sl], w1_tiles[0][j][:, 0:128],
                                     xbf[:, sl],
                                     start=(j == 0), stop=(j == NCH - 1))
            for j in range(NCH):
                xt = px.tile([128, T], F32, tag="xt", bufs=16, name=f"xt{j}")
                nc.sync.dma_start(out=xt[:], in_=xT_d[j * 128:(j + 1) * 128, :])
                xts.append(xt)
            emit_wave_dma(1)

            # ---- LN stat rows, [1, 1024]: halves gathered into free dim ----
            mu_row = scr_tile("mu_row", (1, T))
            var_row = scr_tile("var_row", (1, T))
            tmp_row = scr_tile("tmp_row", (1, T))
            murstd_row = scr_tile("murstd_row", (1, T))
            lg_sb0 = pc.tile([V + 1, 512], F32, name="lg_sb0")
            nc.vector.tensor_copy(lg_sb0[:], st_ps[0:V + 1, :])
            lg_sb1 = pc.tile([V + 1, 512], F32, name="lg_sb1")
            nc.vector.tensor_copy(lg_sb1[:], st_ps2[0:V + 1, :])
            # engines can only address partitions starting at 0/32/64/96, so
            # hop the sum-x rows (partition 10) to partition 0 via tiny DMAs.
            sx_row = pc.tile([1, T], F32, name="sx_row")
            nc.gpsimd.dma_start(out=sx_row[0:1, 0:512], in_=lg_sb0[V:V + 1, :])
            nc.gpsimd.dma_start(out=sx_row[0:1, 512:], in_=lg_sb1[V:V + 1, :])
            nc.vector.tensor_scalar_mul(mu_row[0:1, :], sx_row[0:1, :], 1.0 / H)
            nc.vector.tensor_scalar_mul(var_row[0:1, 0:512], st_ps[32:33, :],
                                        1.0 / H)
            nc.vector.tensor_scalar_mul(var_row[0:1, 512:], st_ps2[32:33, :],
                                        1.0 / H)
            nc.vector.tensor_mul(tmp_row[0:1, :], mu_row[0:1, :],
                                 mu_row[0:1, :])
            nc.vector.tensor_sub(var_row[0:1, :], var_row[0:1, :],
                                 tmp_row[0:1, :])
            # rstd = (var+eps)^-0.5 via exp(-0.5*ln(.)) on ScalarE (~1e-5 rel);
            # DVE reciprocal is Newton-iterative: ~8us for rows this deep.
            nc.scalar.activation(var_row[0:1, :], var_row[0:1, :], AF.Ln,
                                 bias=eps_ln[:])
            nc.scalar.activation(var_row[0:1, :], var_row[0:1, :], AF.Exp,
                                 bias=0.0, scale=-0.5)
            nc.vector.tensor_mul(murstd_row[0:1, :], mu_row[0:1, :],
                                 var_row[0:1, :])

            # ---- emission helpers ----
            def emit_bc():
                # broadcast rstd / mu*rstd rows into [128, T] via K=1 matmuls
                for t in range(TB):
                    sl = slice(t * 512, (t + 1) * 512)
                    bc_r = bp_psum(f"bc_r{t}")
                    nc.tensor.matmul(bc_r[:], ones_f32[:],
                                     var_row[0:1, sl], start=True, stop=True)
                    nc.scalar.activation(rstd_bc[:, sl], bc_r[:], AF.Copy)
                    bc_m = bp_psum(f"bc_m{t}")
                    nc.tensor.matmul(bc_m[:], ones_f32[:],
                                     murstd_row[0:1, sl], start=True, stop=True)
                    nc.scalar.activation(murstd_bc[:, sl], bc_m[:], AF.Copy)

            def emit_main_tile(j):
                w, jj = j // 4, j % 4
                acc = acc_psum(f"acc{j}")
                for t in range(TB):
                    sl = slice(t * 512, (t + 1) * 512)
                    for hin in range(NCH):
                        nc.tensor.matmul(
                            acc[:, sl],
                            w1_tiles[w][hin][:, jj * 128:(jj + 1) * 128],
                            xbfs[hin][:, sl],
                            start=(hin == 0), stop=(hin == NCH - 1))
                accs[j] = acc

            def emit_evict1(j):
                # xt[j] = psum*rstd + (-r1_j)*murstd + xt[j]
                s3 = scr_tile(f"s3_{j}")
                nc.vector.scalar_tensor_tensor(
                    out=s3[:], in0=accs.pop(j)[:], scalar=1.0, in1=rstd_bc[:],
                    op0=OP.mult, op1=OP.mult)
                s4 = scr_tile(f"s4_{j}")
                nc.vector.scalar_tensor_tensor(
                    out=s4[:], in0=murstd_bc[:], scalar=nr1_sb[:, j:j + 1],
                    in1=s3[:], op0=OP.mult, op1=OP.add)
                nc.vector.tensor_add(xts[j][:], s4[:], xts[j][:])

            bp = {}

            def emit_ev():
                # token-reduce of the phase-A logits rows against rstd
                ev_acc = pc.tile([V, TB], F32)
                for t, lg_sb in ((0, lg_sb0), (1, lg_sb1)):
                    sl = slice(t * 512, (t + 1) * 512)
                    lg_scr = pc.tile([V, 512], F32, tag="lg", bufs=2,
                                     name=f"lg_scr{t}")
                    nc.vector.scalar_tensor_tensor(
                        out=lg_scr[:], in0=lg_sb[0:V, :], scalar=1.0,
                        in1=rstd_bc[0:V, sl], op0=OP.mult, op1=OP.mult,
                        accum_out=ev_acc[:, t:t + 1])
                rv_scr = pc.tile([V, T], F32)
                rv_acc = pc.tile([V, 1], F32)
                nc.vector.tensor_scalar(rv_scr[:], murstd_bc[0:V, :],
                                        rve_sb[:, 0:1], None, op0=OP.mult,
                                        op1=OP.add, accum_out=rv_acc[:])
                ev_sb = pc.tile([V, 1], F32)
                nc.vector.tensor_add(ev_sb[:], ev_acc[:, 0:1], ev_acc[:, 1:2])
                nc.vector.tensor_sub(ev_sb[:], ev_sb[:], rv_acc[:])

                cc_in = pdram.tile([V, 1], F32)
                cc_out = pdram.tile([V, 1], F32)
                nc.gpsimd.dma_start(out=cc_in[:], in_=ev_sb[:])
                nc.gpsimd.collective_compute(
                    "AllReduce", OP.add,
                    replica_groups=[[0, 1], [2, 3], [4, 5], [6, 7]],
                    ins=[cc_in.opt()], outs=[cc_out.opt()])
                cc_sb = pc.tile([V, 1], F32)
                nc.gpsimd.dma_start(out=cc_sb[:], in_=cc_out[:])
                bp["cc"] = cc_sb

            def emit_bp_pre():
                cc_sb = bp["cc"]
                ev0 = pc.tile([V, 1], F32)
                nc.scalar.activation(ev0[:], cc_sb[:], AF.Sigmoid,
                                     bias=bve_sb[:], scale=1.0 / S)
                m1 = pc.tile([V, 1], F32)
                nc.vector.tensor_scalar(m1[:], ev0[:], 0.1, None, op0=OP.is_gt)
                mask = pc.tile([V, 1], F32)
                nc.vector.tensor_scalar(mask[:], ev0[:], 0.9, None,
                                        op0=OP.is_lt)
                nc.vector.tensor_mul(mask[:], mask[:], m1[:])
                nc.vector.tensor_scalar(mask[:], mask[:], hasp_sb[:, 0:1],
                                        None, op0=OP.mult)
                cn_scr = pc.tile([V, D4], F32)
                icn = pc.tile([V, 1], F32)
                nc.vector.scalar_tensor_tensor(
                    out=cn_scr[:], in0=cpt_sb[:], scalar=1.0, in1=cpt_sb[:],
                    op0=OP.mult, op1=OP.mult, accum_out=icn[:])
                nc.scalar.activation(icn[:], icn[:], AF.Sqrt, bias=0.0)
                nc.vector.reciprocal(icn[:], icn[:])
                probs = pc.tile([V, 1], F32)
                nc.vector.tensor_copy(probs[:], ev0[:])
                bp.update(mask=mask, icn=icn, probs=probs)

            def emit_bp_iter(it):
                mask, icn, probs = bp["mask"], bp["icn"], bp["probs"]
                lhsT = pc.tile([V, V], BF16, name=f"lhsT{it}")
                nc.vector.tensor_scalar(lhsT[:], pft_sb[:], probs[:, 0:1],
                                        None, op0=OP.mult)
                pe_ps = bp_psum(f"pe{it}")
                nc.tensor.matmul(pe_ps[0:V, 0:D4], lhsT[:], var_sb[:],
                                 start=True, stop=True)
                pe_sb = pc.tile([V, D4], F32, tag="bscr", bufs=3,
                                name=f"pe_sb{it}")
                nc.vector.tensor_copy(pe_sb[:], pe_ps[0:V, 0:D4])
                bscr = pc.tile([V, D4], F32, tag="bscr", bufs=3,
                               name=f"bscr{it}")
                dot = pc.tile([V, 1], F32, name=f"dot{it}")
                nc.vector.scalar_tensor_tensor(
                    out=bscr[:], in0=pe_sb[:], scalar=1.0, in1=cpt_sb[:],
                    op0=OP.mult, op1=OP.mult, accum_out=dot[:])
                bscr2 = pc.tile([V, D4], F32, tag="bscr", bufs=3,
                                name=f"bscr2{it}")
                sqn = pc.tile([V, 1], F32, name=f"sqn{it}")
                nc.vector.scalar_tensor_tensor(
                    out=bscr2[:], in0=pe_sb[:], scalar=1.0, in1=pe_sb[:],
                    op0=OP.mult, op1=OP.mult, accum_out=sqn[:])
                # +1e-16 keeps parentless rows (pe == 0) finite; they are
                # masked out of the update anyway (matches the 1e-8 clamp).
                nc.scalar.activation(sqn[:], sqn[:], AF.Sqrt, bias=eps_pn[:])
                ipn = pc.tile([V, 1], F32, name=f"ipn{it}")
                nc.vector.reciprocal(ipn[:], sqn[:])
                s = pc.tile([V, 1], F32, name=f"s{it}")
                nc.vector.tensor_mul(s[:], dot[:], ipn[:])
                nc.vector.tensor_mul(s[:], s[:], icn[:])
                cond = pc.tile([V, 1], F32, name=f"cond{it}")
                nc.scalar.activation(cond[:], s[:], AF.Sigmoid, bias=0.0)
                delta = pc.tile([V, 1], F32, name=f"delta{it}")
                nc.vector.tensor_sub(delta[:], cond[:], probs[:])
                nc.vector.tensor_mul(delta[:], delta[:], mask[:])
                nc.vector.tensor_add(probs[:], probs[:], delta[:])

            def emit_ccol():
                probs = bp["probs"]
                ccol_ps = bp_psum("ccol_ps")
                for c in range(NCH):
                    nc.tensor.matmul(ccol_ps[:, c:c + 1],
                                     w2t_sb[:, c * 128:(c + 1) * 128],
                                     probs[:], start=True, stop=True)
                ccol_sb = pc.tile([128, NCH], F32)
                nc.vector.tensor_add(ccol_sb[:], ccol_ps[:, 0:NCH], bout_sb[:])
                bp["ccol"] = ccol_sb

            def emit_evict2(j):
                nc.vector.tensor_scalar(xts[j][:], xts[j][:],
                                        bp["ccol"][:, j:j + 1], None,
                                        op0=OP.add)
                nc.sync.dma_start(out=out_d[j * 128:(j + 1) * 128, :],
                                  in_=xts[j][:])

            # ---- main emission schedule (tile 0 already ran in phase A) ----
            for j in range(1, NCH):
                if j == 2:
                    emit_wave_dma(2)
                if j == 6:
                    emit_wave_dma(3)
                emit_main_tile(j)
                if j == 2:
                    emit_bc()
                    emit_ev()
                    emit_evict1(0)
                    emit_evict1(1)
                elif 3 <= j <= 14:
                    emit_evict1(j - 1)
                if j == 6:
                    emit_bp_pre()
                if 7 <= j <= 11:
                    emit_bp_iter(j - 7)
                if j == 12:
                    emit_ccol()
                if j == 13:
                    for jj in range(13):
                        emit_evict2(jj)
                if j == 14:
                    emit_evict1(14)
                    emit_evict2(13)
                if j == 15:
                    emit_evict2(14)
                    emit_evict1(15)
                    emit_evict2(15)

    nc.compile()
    return nc


def _host_prep(hidden_states, gamma, beta, W_ve, b_ve, var_emb, cpt_emb,
               W_out, b_out, parents):
    f32 = np.float32
    x = np.asarray(hidden_states, f32).reshape(B * S, H)
    gamma = np.asarray(gamma, f32)
    beta = np.asarray(beta, f32)
    W_ve = np.asarray(W_ve, f32)
    b_ve = np.asarray(b_ve, f32)
    var_emb = np.asarray(var_emb, f32)
    cpt_emb = np.asarray(cpt_emb, f32)
    W_out = np.asarray(W_out, f32)
    b_out = np.asarray(b_out, f32)
    parents = np.asarray(parents)

    W1 = W_out[:, :H]
    W1g = W1 * gamma[None, :]
    w1t = np.ascontiguousarray(W1g.T).astype(ml_dtypes.bfloat16)
    w2t = np.ascontiguousarray(W_out[:, H:].T)
    Wveg = W_ve * gamma[None, :]
    wve_t = np.ascontiguousarray(
        np.concatenate([Wveg.T, np.ones((H, 1), f32)], axis=1)
    ).astype(ml_dtypes.bfloat16)
    rve_col = Wveg.sum(axis=1).reshape(V, 1).astype(f32)
    bve_col = (b_ve + W_ve @ beta).reshape(V, 1).astype(f32)
    hasp_col = (parents.sum(axis=1) > 0).astype(f32).reshape(V, 1)
    pft = np.ascontiguousarray(parents.T.astype(f32))
    var_bf = var_emb.astype(ml_dtypes.bfloat16)
    cpt = np.ascontiguousarray(cpt_emb, f32)
    neg_r1 = np.ascontiguousarray((-W1g.sum(axis=1)).reshape(NCH, 128).T, f32)
    bout_col = np.ascontiguousarray(
        (b_out + W1 @ beta).reshape(NCH, 128).T, f32)

    shared = dict(w1t=w1t, w2t=w2t, wve_t=wve_t, rve_col=rve_col,
                  bve_col=bve_col, hasp_col=hasp_col, pft=pft, var_bf=var_bf,
                  cpt=cpt, neg_r1=neg_r1, bout_col=bout_col)
    in_maps = []
    for c in range(N_CORES):
        xT = np.ascontiguousarray(x[c * T:(c + 1) * T, :].T)
        in_maps.append(dict(shared, xT=xT,
                            xbfT=xT.astype(ml_dtypes.bfloat16)))
    return in_maps


def kernel(**inputs):
    global _PROG
    if _PROG is None:
        _PROG = build_program()
    nc = _PROG
    in_maps = _host_prep(**inputs)
    res = run_bass_kernel_spmd(nc, in_maps, list(range(N_CORES)))
    out = np.empty((B * S, H), np.float32)
    for c in range(N_CORES):
        out[c * T:(c + 1) * T, :] = res.results[c]["outT"].T
    return out.reshape(B, S, H)
